# revision 4
# baseline (speedup 1.0000x reference)
"""3D Haar DWT (depth-1) Trainium2 kernel — bf16 pipeline.

Full inputs: x [4, 4, 64, 256, 256] f32 + six banded Haar matrices
(hardcoded math: every output element is +-2^-1.5 times a +-sum of a
2x2x2 block). Returns the 8 subbands (LLL, LLH, LHL, LHH, HLL, HLH,
HHL, HHH), each [4, 4, 32, 128, 128] f32.

Tolerance is 2e-2 max-abs-relative, so the whole pipeline runs in
bf16 on device (input cast + output cast happen on host): DMA traffic
halves to 33.6 MB/core and DVE tensor ops hit the 2x_1P perf mode.

Sharding: data-parallel over N*C = 16 sample-channels, 2 per core on
8 cores. Per-core compute processes KG=2 d-pairs (4 depth slices) per
step, pair-packed so every load descriptor is a 1 KiB linear run:
  H stage: row pairs    -> TensorE matmuls against +-2^-1.5 * I; the
                           moving APs split w into (even, odd) halves
                           so PSUM comes out parity-deinterleaved.
  evac:    ScalarE copies PSUM (fp32) -> SBUF (bf16).
  W stage: parity halves-> DVE tensor_add/sub, unit-stride, 2x mode
  D stage: slice pairs  -> DVE tensor_add/sub, unit-stride, 2x mode
Loads issue on the Sync HWDGE ring, stores on the Scalar HWDGE ring so
the two streams drain independently (no store backlog behind loads).
"""
import sys

sys.path.insert(0, "/opt/trn_rl_repo")

import numpy as np
import ml_dtypes

N, C, D, H, W = 4, 4, 64, 256, 256
NCORES = 8
G_PER_CORE = (N * C) // NCORES        # 2
KP = D // 2                           # 32 d-pairs per g
KG = 2                                # d-pairs per step
NSTEP = KP // KG                      # 16 steps per g
S3 = float(2.0 ** -1.5)
BF16 = ml_dtypes.bfloat16

# schedule tunables
IN_BUFS = 4
EV_BUFS = 4
WT_BUFS = 4
OS_BUFS = 4
PSUM_BUFS = 2

_CACHE = {}


def _build_filter_lhst():
    """Stationary operands: +S3*I and -S3*I, as [2, 128, 128] bf16."""
    eye = np.eye(128, dtype=np.float32)
    return np.stack([np.float32(S3) * eye,
                     np.float32(-S3) * eye]).astype(BF16)


def _build_nc():
    import concourse.tile as tile
    from concourse import bacc, mybir

    f32 = mybir.dt.float32
    bf16 = mybir.dt.bfloat16
    FD = KG * 1024                    # free-dim elems per step tile
    nc = bacc.Bacc(None)
    x_d = nc.declare_dram_parameter("x", [G_PER_CORE, D, H, W], bf16,
                                    isOutput=False)
    ft_d = nc.declare_dram_parameter("ft", [2, 128, 128], bf16,
                                     isOutput=False)
    # per (g, partition=h', step): one contiguous 4 KiB run holding
    # [band, kq, u] (host transposes back to subband-major)
    o_d = nc.declare_dram_parameter("out", [G_PER_CORE, 128, NSTEP, 8, KG, 128],
                                    bf16, isOutput=True)

    with tile.TileContext(nc) as tc:
        with (
            tc.tile_pool(name="cst", bufs=1) as cst,
            tc.tile_pool(name="inp", bufs=IN_BUFS) as inp,
            tc.tile_pool(name="ev", bufs=EV_BUFS) as evp,
            tc.tile_pool(name="wt", bufs=WT_BUFS) as wtp,
            tc.tile_pool(name="os", bufs=OS_BUFS) as osp,
            tc.tile_pool(name="ps", bufs=PSUM_BUFS, space="PSUM") as psp,
        ):
            ft = cst.tile([128, 256], bf16, tag="ft")
            nc.scalar.dma_start(
                ft.rearrange("p (i c) -> p i c", i=2),
                ft_d.rearrange("i p c -> p i c"))
            pos_i = ft[:, 0:128]    # +S3 * I
            neg_i = ft[:, 128:256]  # -S3 * I

            for g in range(G_PER_CORE):
                for st in range(NSTEP):
                    # 512 KiB load: KG d-pairs, pair-packed [k, s, r, w]
                    blk = inp.tile([128, FD], bf16, tag="xin")
                    nc.sync.dma_start(
                        blk.rearrange("p (k s r) -> p k s r", k=KG, s=2),
                        x_d[g, st * 2 * KG:(st + 1) * 2 * KG].rearrange(
                            "(k s) (p r) w -> p k s (r w)", s=2, r=2))
                    # w = 2u + par; moving APs iterate (k, s, par, u)
                    mov = blk.rearrange("p (k s r u par) -> p r k s par u",
                                        k=KG, s=2, r=2, par=2)
                    # --- H stage on TensorE: psum [b, k, s, par, u] ---
                    pt = psp.tile([128, FD], f32, tag="ps")
                    pt3 = pt.rearrange("p (b k s par u) -> p b k s par u",
                                       b=2, k=KG, s=2, par=2)
                    # one matmul output = one PSUM bank (512 fp32)
                    for b in range(2):
                        for k in range(KG):
                            nc.tensor.matmul(pt3[:, b, k], pos_i,
                                             mov[:, 0, k],
                                             start=True, stop=False)
                            nc.tensor.matmul(pt3[:, b, k],
                                             pos_i if b == 0 else neg_i,
                                             mov[:, 1, k],
                                             start=False, stop=True)
                    # --- ScalarE evacuation: fp32 -> bf16 ---
                    ev = evp.tile([128, FD], bf16, tag="ev")
                    nc.scalar.activation(
                        ev[:], pt[:], mybir.ActivationFunctionType.Copy)
                    ev4 = ev.rearrange("p (b k s par u) -> p b k s par u",
                                       b=2, k=KG, s=2, par=2)
                    # --- W stage on DVE (unit stride, 2x) ---
                    wt_t = wtp.tile([128, FD], bf16, tag="wt")
                    wt4 = wt_t.rearrange("p (b k s q u) -> p b k s q u",
                                         b=2, k=KG, s=2, q=2)
                    nc.vector.tensor_add(wt4[:, :, :, :, 0],
                                         ev4[:, :, :, :, 0],
                                         ev4[:, :, :, :, 1])
                    nc.vector.tensor_sub(wt4[:, :, :, :, 1],
                                         ev4[:, :, :, :, 0],
                                         ev4[:, :, :, :, 1])
                    # --- D stage on DVE (unit stride, 2x) ---
                    # os layout [dd, b, q, kq, u]; iterate (kq, b, q, u)
                    os_t = osp.tile([128, FD], bf16, tag="os")
                    osd = os_t.rearrange("p (dd b q k u) -> p dd k b q u",
                                         dd=2, b=2, q=2, k=KG)
                    wtd = wt_t.rearrange("p (b k s q u) -> p s k b q u",
                                         b=2, k=KG, s=2, q=2)
                    nc.vector.tensor_add(osd[:, 0], wtd[:, 0], wtd[:, 1])
                    nc.vector.tensor_sub(osd[:, 1], wtd[:, 0], wtd[:, 1])
                    # 512 KiB store: 4 KiB contiguous per partition
                    nc.scalar.dma_start(
                        o_d[g, :, st],
                        os_t.rearrange("p (band kq u) -> p band kq u",
                                       band=8, kq=KG))
    nc.finalize()
    return nc


def _get_nc():
    if "nc" not in _CACHE:
        _CACHE["nc"] = _build_nc()
    return _CACHE["nc"]


def _make_in_maps(x):
    xs = np.asarray(x, dtype=np.float32).reshape(N * C, D, H, W).astype(BF16)
    ft = _build_filter_lhst()
    return [
        {"x": np.ascontiguousarray(
            xs[c * G_PER_CORE:(c + 1) * G_PER_CORE]), "ft": ft}
        for c in range(NCORES)
    ]


def _unshard(core_outs):
    """core_outs[c]: [G, 128, NSTEP, 8, KG, 128] bf16 -> 8 full f32 bands."""
    full = np.empty((8, N * C, KP, 128, 128), dtype=np.float32)
    for c, arr in enumerate(core_outs):
        a = np.asarray(arr).astype(np.float32)
        a = a.transpose(3, 0, 2, 4, 1, 5).reshape(8, G_PER_CORE, KP, 128, 128)
        full[:, c * G_PER_CORE:(c + 1) * G_PER_CORE] = a
    full = full.reshape(8, N, C, KP, 128, 128)
    return tuple(full[s] for s in range(8))


def kernel(x, low_0, low_1, low_2, high_0, high_1, high_2):
    from concourse.bass_utils import run_bass_kernel_spmd

    in_maps = _make_in_maps(x)
    nc = _get_nc()
    res = run_bass_kernel_spmd(nc, in_maps, list(range(NCORES)))
    return _unshard([res.results[c]["out"] for c in range(NCORES)])


# revision 5
# speedup vs baseline: 1.0931x; 1.0931x over previous
"""3D Haar DWT (depth-1) Trainium2 kernel — bf16 pipeline.

Full inputs: x [4, 4, 64, 256, 256] f32 + six banded Haar matrices
(hardcoded math: every output element is +-2^-1.5 times a +-sum of a
2x2x2 block). Returns the 8 subbands (LLL, LLH, LHL, LHH, HLL, HLH,
HHL, HHH), each [4, 4, 32, 128, 128] f32.

Tolerance is 2e-2 max-abs-relative, so the whole pipeline runs in
bf16 on device (input cast + output cast happen on host): DMA traffic
halves to 33.6 MB/core and DVE tensor ops hit the 2x_1P perf mode.

Sharding: data-parallel over N*C = 16 sample-channels, 2 per core on
8 cores. Per-core compute, KG=2 d-pairs (4 depth slices) per step:
  H stage: TensorE, ONE matmul per 512-col chunk against the block
           stationary S3 * (I_64 (x) [[1,1],[1,-1]]) — partition p is
           a raw h row, so output partition 2i/2i+1 = lo/hi of pair i.
           Moving APs order the free dim (par, s, hh, k, u) so PSUM
           lands with both later butterfly dims outermost.
  evac:    ScalarE copies PSUM (fp32) -> SBUF (bf16), contiguous.
  W stage: DVE add/sub over the two w-parity halves — single-run APs.
  D stage: DVE add/sub over the two slice halves — 512-elem runs.
Loads issue on the Sync HWDGE ring, stores on the GpSimd SWDGE ring so
the three streams drain independently. Per-partition runs: loads 512 B,
stores 4 KiB.
"""
import sys

sys.path.insert(0, "/opt/trn_rl_repo")

import numpy as np
import ml_dtypes

N, C, D, H, W = 4, 4, 64, 256, 256
NCORES = 8
G_PER_CORE = (N * C) // NCORES        # 2
KP = D // 2                           # 32 d-pairs per g
KG = 2                                # d-pairs per step
NSTEP = KP // KG                      # 16 steps per g
S3 = float(2.0 ** -1.5)
BF16 = ml_dtypes.bfloat16

# schedule tunables
IN_BUFS = 6
EV_BUFS = 4
WT_BUFS = 4
OS_BUFS = 6
PSUM_BUFS = 2

_CACHE = {}


def _build_filter_lhst():
    """Stationary operand: S3 * (I_64 kron [[1,1],[1,-1]]), [128,128] bf16."""
    had = np.array([[1.0, 1.0], [1.0, -1.0]], dtype=np.float32)
    s = np.kron(np.eye(64, dtype=np.float32), had) * np.float32(S3)
    return s.astype(BF16)


def _build_nc():
    import concourse.tile as tile
    from concourse import bacc, mybir

    f32 = mybir.dt.float32
    bf16 = mybir.dt.bfloat16
    FD = KG * 1024                    # free-dim elems per step tile
    nc = bacc.Bacc(None)
    x_d = nc.declare_dram_parameter("x", [G_PER_CORE, D, H, W], bf16,
                                    isOutput=False)
    ft_d = nc.declare_dram_parameter("ft", [128, 128], bf16,
                                     isOutput=False)
    # per (g, partition, step): one contiguous 4 KiB run holding
    # [dd, q, hh, k, u] (host transposes back to subband-major)
    o_d = nc.declare_dram_parameter(
        "out", [G_PER_CORE, 128, NSTEP, 2, 2, 2, KG, 128], bf16,
        isOutput=True)

    with tile.TileContext(nc) as tc:
        with (
            tc.tile_pool(name="cst", bufs=1) as cst,
            tc.tile_pool(name="inp", bufs=IN_BUFS) as inp,
            tc.tile_pool(name="ev", bufs=EV_BUFS) as evp,
            tc.tile_pool(name="wt", bufs=WT_BUFS) as wtp,
            tc.tile_pool(name="os", bufs=OS_BUFS) as osp,
            tc.tile_pool(name="ps", bufs=PSUM_BUFS, space="PSUM") as psp,
        ):
            ft = cst.tile([128, 128], bf16, tag="ft")
            nc.scalar.dma_start(ft[:], ft_d[:])

            for g in range(G_PER_CORE):
                for st in range(NSTEP):
                    # 512 KiB load: KG d-pairs; partition p = h row;
                    # free dim (k, s, hh, w), 512 B runs
                    blk = inp.tile([128, FD], bf16, tag="xin")
                    nc.sync.dma_start(
                        blk.rearrange("p (k s hh w) -> p k s hh w",
                                      k=KG, s=2, hh=2),
                        x_d[g, st * 2 * KG:(st + 1) * 2 * KG].rearrange(
                            "(k s) (hh p) w -> p k s hh w", s=2, hh=2))
                    # --- H stage on TensorE: psum (par, s, hh, k, u) ---
                    pt = psp.tile([128, FD], f32, tag="ps")
                    pt5 = pt.rearrange("p (par s hh k u) -> p par s hh k u",
                                       par=2, s=2, hh=2, k=KG)
                    mov = blk.rearrange("p (k s hh u par) -> p par s hh k u",
                                        k=KG, s=2, hh=2, par=2)
                    for par in range(2):
                        for s in range(2):
                            nc.tensor.matmul(pt5[:, par, s], ft[:],
                                             mov[:, par, s],
                                             start=True, stop=True)
                    # --- ScalarE evacuation: fp32 -> bf16, contiguous ---
                    ev = evp.tile([128, FD], bf16, tag="ev")
                    nc.scalar.activation(
                        ev[:], pt[:], mybir.ActivationFunctionType.Copy)
                    ev5 = ev.rearrange("p (par s hh k u) -> p par s hh k u",
                                       par=2, s=2, hh=2, k=KG)
                    # --- W stage on DVE: single-run APs, 2x mode ---
                    wt_t = wtp.tile([128, FD], bf16, tag="wt")
                    wt5 = wt_t.rearrange("p (q s hh k u) -> p q s hh k u",
                                         q=2, s=2, hh=2, k=KG)
                    nc.vector.tensor_add(wt5[:, 0], ev5[:, 0], ev5[:, 1])
                    nc.vector.tensor_sub(wt5[:, 1], ev5[:, 0], ev5[:, 1])
                    # --- D stage on DVE: 512-elem runs in, 1-run out ---
                    os_t = osp.tile([128, FD], bf16, tag="os")
                    os5 = os_t.rearrange("p (dd q hh k u) -> p dd q hh k u",
                                         dd=2, q=2, hh=2, k=KG)
                    wtd = wt_t.rearrange("p (q s hh k u) -> p s q hh k u",
                                         q=2, s=2, hh=2, k=KG)
                    nc.vector.tensor_add(os5[:, 0], wtd[:, 0], wtd[:, 1])
                    nc.vector.tensor_sub(os5[:, 1], wtd[:, 0], wtd[:, 1])
                    # 512 KiB store on the SWDGE ring, 4 KiB runs
                    nc.gpsimd.dma_start(
                        o_d[g, :, st].rearrange(
                            "p dd q hh k u -> p (dd q hh k u)"),
                        os_t[:])
    nc.finalize()
    return nc


def _get_nc():
    if "nc" not in _CACHE:
        _CACHE["nc"] = _build_nc()
    return _CACHE["nc"]


def _make_in_maps(x):
    xs = np.asarray(x, dtype=np.float32).reshape(N * C, D, H, W).astype(BF16)
    ft = _build_filter_lhst()
    return [
        {"x": np.ascontiguousarray(
            xs[c * G_PER_CORE:(c + 1) * G_PER_CORE]), "ft": ft}
        for c in range(NCORES)
    ]


def _unshard(core_outs):
    """core_outs[c]: [G, 128, NSTEP, 2, 2, 2, KG, 128] bf16 -> 8 bands.

    band = 4*dd + 2*(p%2) + q; h' = hh*64 + p//2; d' = st*KG + k.
    """
    full = np.empty((8, N * C, KP, 128, 128), dtype=np.float32)
    for c, arr in enumerate(core_outs):
        a = np.asarray(arr).astype(np.float32)
        a = a.reshape(G_PER_CORE, 64, 2, NSTEP, 2, 2, 2, KG, 128)
        #            g, p2, pb, st, dd, q, hh, k, u
        a = a.transpose(4, 2, 5, 0, 3, 7, 6, 1, 8)
        #            dd, pb, q, g, st, k, hh, p2, u
        a = a.reshape(8, G_PER_CORE, KP, 128, 128)
        full[:, c * G_PER_CORE:(c + 1) * G_PER_CORE] = a
    full = full.reshape(8, N, C, KP, 128, 128)
    return tuple(full[s] for s in range(8))


def kernel(x, low_0, low_1, low_2, high_0, high_1, high_2):
    from concourse.bass_utils import run_bass_kernel_spmd

    in_maps = _make_in_maps(x)
    nc = _get_nc()
    res = run_bass_kernel_spmd(nc, in_maps, list(range(NCORES)))
    return _unshard([res.results[c]["out"] for c in range(NCORES)])


# revision 6
# speedup vs baseline: 1.1398x; 1.0426x over previous
"""3D Haar DWT (depth-1) Trainium2 kernel — bf16 pipeline.

Full inputs: x [4, 4, 64, 256, 256] f32 + six banded Haar matrices
(hardcoded math: every output element is +-2^-1.5 times a +-sum of a
2x2x2 block). Returns the 8 subbands (LLL, LLH, LHL, LHH, HLL, HLH,
HHL, HHH), each [4, 4, 32, 128, 128] f32.

Tolerance is 2e-2 max-abs-relative, so the whole pipeline runs in
bf16 on device (casts happen on host): DMA traffic halves to
33.6 MB/core and DVE tensor ops hit the 2x_1P perf mode.

The host also PRE-PACKS x into the exact per-step SBUF tile layout
(and un-packs the output), so every DMA moves one contiguous 4 KiB
run per partition — descriptor generation is no longer a bottleneck.

Sharding: data-parallel over N*C = 16 sample-channels, 2 per core on
8 cores. Per-core compute, KG=2 d-pairs (4 depth slices) per step;
free-dim layout (k, s, hh, par, u), partition p = h row mod 128:
  H+D:  TensorE. Stationary S3 * (I_64 (x) [[1,1],[1,-1]]) performs
        the H butterfly across partition pairs; the +-Hadamard pair
        accumulated over the two depth slices (s) performs the D
        butterfly in PSUM. psum layout (par, dd, hh, k, u).
  evac: ScalarE copies PSUM (fp32) -> SBUF (bf16), contiguous.
  W:    DVE add/sub of the two contiguous w-parity halves — flat
        single-run APs at 2 elem/cycle.
Loads issue on the Sync HWDGE ring, stores on the GpSimd SWDGE ring.
"""
import sys

sys.path.insert(0, "/opt/trn_rl_repo")

import numpy as np
import ml_dtypes

N, C, D, H, W = 4, 4, 64, 256, 256
NCORES = 8
G_PER_CORE = (N * C) // NCORES        # 2
KP = D // 2                           # 32 d-pairs per g
KG = 2                                # d-pairs per step
NSTEP = KP // KG                      # 16 steps per g
S3 = float(2.0 ** -1.5)
BF16 = ml_dtypes.bfloat16

# schedule tunables
IN_BUFS = 6
EV_BUFS = 6
OS_BUFS = 6
PSUM_BUFS = 2

_CACHE = {}


def _build_filter_lhst():
    """Stationaries +-S3 * (I_64 kron [[1,1],[1,-1]]), [2, 128, 128] bf16."""
    had = np.array([[1.0, 1.0], [1.0, -1.0]], dtype=np.float32)
    s = np.kron(np.eye(64, dtype=np.float32), had) * np.float32(S3)
    return np.stack([s, -s]).astype(BF16)


def _build_nc():
    import concourse.tile as tile
    from concourse import bacc, mybir

    f32 = mybir.dt.float32
    bf16 = mybir.dt.bfloat16
    FD = KG * 1024                    # free-dim elems per step tile
    nc = bacc.Bacc(None)
    # host-packed: [g, st, p, (k, s, hh, par, u)]
    x_d = nc.declare_dram_parameter("x", [G_PER_CORE, NSTEP, 128, FD], bf16,
                                    isOutput=False)
    ft_d = nc.declare_dram_parameter("ft", [2, 128, 128], bf16,
                                     isOutput=False)
    # per (g, p, st): one contiguous 4 KiB run holding (q, dd, hh, k, u)
    o_d = nc.declare_dram_parameter("out", [G_PER_CORE, 128, NSTEP, FD], bf16,
                                    isOutput=True)

    with tile.TileContext(nc) as tc:
        with (
            tc.tile_pool(name="cst", bufs=1) as cst,
            tc.tile_pool(name="inp", bufs=IN_BUFS) as inp,
            tc.tile_pool(name="ev", bufs=EV_BUFS) as evp,
            tc.tile_pool(name="os", bufs=OS_BUFS) as osp,
            tc.tile_pool(name="ps", bufs=PSUM_BUFS, space="PSUM") as psp,
        ):
            ft = cst.tile([128, 256], bf16, tag="ft")
            nc.scalar.dma_start(ft.rearrange("p (i c) -> p i c", i=2),
                                ft_d.rearrange("i p c -> p i c"))
            had_p = ft[:, 0:128]    # +S3 * (I (x) Hadamard)
            had_n = ft[:, 128:256]  # negated

            for g in range(G_PER_CORE):
                for st in range(NSTEP):
                    blk = inp.tile([128, FD], bf16, tag="xin")
                    nc.sync.dma_start(blk[:], x_d[g, st])
                    # moving views: (s, par) -> (hh, k, u), contiguous u
                    mov = blk.rearrange("p (k s hh par u) -> p s par hh k u",
                                        k=KG, s=2, hh=2, par=2)
                    # --- H (stationary) + D (psum accumulation) ---
                    pt = psp.tile([128, FD], f32, tag="ps")
                    pt5 = pt.rearrange("p (par dd hh k u) -> p par dd hh k u",
                                       par=2, dd=2, hh=2, k=KG)
                    for par in range(2):
                        for dd in range(2):
                            nc.tensor.matmul(pt5[:, par, dd], had_p,
                                             mov[:, 0, par],
                                             start=True, stop=False)
                            nc.tensor.matmul(pt5[:, par, dd],
                                             had_p if dd == 0 else had_n,
                                             mov[:, 1, par],
                                             start=False, stop=True)
                    # --- ScalarE evacuation: fp32 -> bf16, contiguous ---
                    ev = evp.tile([128, FD], bf16, tag="ev")
                    nc.scalar.activation(
                        ev[:], pt[:], mybir.ActivationFunctionType.Copy)
                    # --- W stage on DVE: flat single-run APs, 2x mode ---
                    os_t = osp.tile([128, FD], bf16, tag="os")
                    half = FD // 2
                    nc.vector.tensor_add(os_t[:, 0:half],
                                         ev[:, 0:half], ev[:, half:FD])
                    nc.vector.tensor_sub(os_t[:, half:FD],
                                         ev[:, 0:half], ev[:, half:FD])
                    # 512 KiB store on the SWDGE ring, 4 KiB runs
                    nc.gpsimd.dma_start(o_d[g, :, st], os_t[:])
    nc.finalize()
    return nc


def _get_nc():
    if "nc" not in _CACHE:
        _CACHE["nc"] = _build_nc()
    return _CACHE["nc"]


def _pack_x(x):
    """[N*C, D, H, W] f32 -> per-core [G, NSTEP, 128, FD] bf16 tiles.

    d = st*2*KG + k*2 + s; h = hh*128 + p; w = 2u + par.
    Free-dim layout per (g, st, p): (k, s, hh, par, u).
    """
    xs = np.asarray(x, dtype=np.float32).reshape(N * C, D, H, W).astype(BF16)
    xs = xs.reshape(N * C, NSTEP, KG, 2, 2, 128, 128, 2)
    #              gc, st, k, s, hh, p, u, par
    xs = xs.transpose(0, 1, 5, 2, 3, 4, 7, 6)
    #              gc, st, p, k, s, hh, par, u
    xs = np.ascontiguousarray(xs.reshape(N * C, NSTEP, 128, KG * 1024))
    return xs


def _make_in_maps(x):
    xs = _pack_x(x)
    ft = _build_filter_lhst()
    return [
        {"x": xs[c * G_PER_CORE:(c + 1) * G_PER_CORE], "ft": ft}
        for c in range(NCORES)
    ]


def _unshard(core_outs):
    """core_outs[c]: [G, 128, NSTEP, FD] bf16 -> 8 full f32 bands.

    Free dim is (q, dd, hh, k, u); band = 4*dd + 2*(p%2) + q;
    h' = hh*64 + p//2; d' = st*KG + k.
    """
    full = np.empty((8, N * C, KP, 128, 128), dtype=np.float32)
    for c, arr in enumerate(core_outs):
        a = np.asarray(arr).astype(np.float32)
        a = a.reshape(G_PER_CORE, 64, 2, NSTEP, 2, 2, 2, KG, 128)
        #            g, p2, pb, st, q, dd, hh, k, u
        a = a.transpose(5, 2, 4, 0, 3, 7, 6, 1, 8)
        #            dd, pb, q, g, st, k, hh, p2, u
        a = a.reshape(8, G_PER_CORE, KP, 128, 128)
        full[:, c * G_PER_CORE:(c + 1) * G_PER_CORE] = a
    full = full.reshape(8, N, C, KP, 128, 128)
    return tuple(full[s] for s in range(8))


def kernel(x, low_0, low_1, low_2, high_0, high_1, high_2):
    from concourse.bass_utils import run_bass_kernel_spmd

    in_maps = _make_in_maps(x)
    nc = _get_nc()
    res = run_bass_kernel_spmd(nc, in_maps, list(range(NCORES)))
    return _unshard([res.results[c]["out"] for c in range(NCORES)])
